# revision 1
# baseline (speedup 1.0000x reference)
"""AdaptedAttention (llama + adaption-prompt) on 8 TRN2 NeuronCores.

Sharding: tensor-parallel over heads (2 heads/core), zero device collectives.
Per core (everything on-chip fp16; PSUM accumulation fp32):
  - qT/kT/V projections for its 2 heads from fp16, pre-transposed X
    (all matmuls contract over d, so X lives on-chip as X.T [d part, s free])
  - RoPE in the [hd, s] layout with HOST-precomputed fp16 cos/sin tables
    (sin pre-signed AND half-swapped so each DVE TensorTensor's two SBUF
    inputs share a base partition -- a HW requirement): one DVE fp16 copy
    from PSUM + 4 fp16 2x-mode DVE multiplies per call
  - attention as S^T = K @ Q^T tiles ([k part, q free]) so softmax needs no
    transposes: exp on ACT (scale folded) to fp16 SBUF, row-sums via
    ones-matmuls into rows 0/32 of a per-head [33, QC] PSUM bank, causal
    handled by skipping k-tiles + 0/1 fp16 masks on diagonal tiles, no
    max-subtraction (scores are O(5) here); score matmuls pre-issued 2 deep
  - adapter path (L=10) folded into the main ctx PSUM accumulation:
    ct = (ctx + actx_g * sum/asum) / sum, gate pre-folded into the
    host-scaled adapter-V copy (aptv). 1/asum stays fp32 (asum ~ 6e5 makes
    it fp16-subnormal); 1/sum and sum/asum are fp16-safe
  - output projection uses only the core's OWN 2-head ct against its
    256-column slice of Wo, producing fp16 partials [d, s]; the cross-core
    sum happens on the host as the unshard step (collectives measured far
    slower than 8x16MB host gathers here)
PSUM banks (8): st x2 (scores/adapter/broadcast scratch), qk x2 (q/k proj +
out-proj accumulators), ctx x2, sums+v x2.
Emission order IS the schedule (engine queues are FIFO): per chunk qc we
emit attention-scores(qc), then proj(qc+1), then the combine pass (whose
DVE reciprocal chain drains while PE runs the projections), then out-proj;
anything else head-of-line-blocks the PE queue. Host side: weight slicing/
transposes/casts, RoPE tables from position_ids, partial-sum + transpose.
"""

import math
import numpy as np

import concourse.bass as bass
import concourse.bacc as bacc
import concourse.mybir as mybir
import concourse.tile as tile
from concourse.bass_utils import run_bass_kernel_spmd

F16 = mybir.dt.float16
BF16 = mybir.dt.bfloat16
F32 = mybir.dt.float32
NP_F16 = mybir.dt.np(F16)
NP_BF16 = mybir.dt.np(BF16)


class Cfg:
    def __init__(self, s=2048, d=2048, L=10, n_cores=8, n_heads=16, rope_base=10000.0):
        self.s, self.d, self.L = s, d, L
        self.n_cores = n_cores
        self.n_heads = n_heads
        self.rope_base = rope_base
        self.hd = 128                      # head dim (fixed)
        self.hpc = n_heads // n_cores      # heads per core
        self.dh = self.hpc * self.hd       # local head-dim cols per core
        self.nd = d // 128                 # contraction chunks
        self.QC = 512                      # q-chunk width
        self.ns = s // self.QC             # q-chunks
        self.nst = s // 128                # s tiles (k tiles)
        self.kpq = self.QC // 128          # k-tiles straddling one q-chunk diag
        assert self.hpc * n_cores == n_heads and d % 128 == 0 and s % self.QC == 0


def build(cfg: Cfg, nrep: int = 1):
    """Build the per-core SPMD graph. Returns compiled nc.
    nrep>1 repeats the whole pipeline (for marginal-time HW measurement)."""
    c = cfg
    nc = bacc.Bacc(None, target_bir_lowering=False, num_devices=c.n_cores)

    # ---------------- external I/O (per-core shards) ----------------
    xt_d = nc.dram_tensor("xt", [c.d, c.s], F16, kind="ExternalInput")
    wqt_d = nc.dram_tensor("wqt", [c.d, c.dh], F16, kind="ExternalInput")
    wkt_d = nc.dram_tensor("wkt", [c.d, c.dh], F16, kind="ExternalInput")
    wvt_d = nc.dram_tensor("wvt", [c.d, c.dh], F16, kind="ExternalInput")
    wot_d = nc.dram_tensor("wot", [c.dh, c.d], F16, kind="ExternalInput")
    apt_d = nc.dram_tensor("apt", [c.d, c.L], F16, kind="ExternalInput")
    aptv_d = nc.dram_tensor("aptv", [c.d, c.L], F16, kind="ExternalInput")
    cos_d = nc.dram_tensor("cost", [128, c.s], F16, kind="ExternalInput")
    sin_d = nc.dram_tensor("sint", [128, c.s], F16, kind="ExternalInput")
    out_d = nc.dram_tensor("out", [c.d, c.s], F16, kind="ExternalOutput")

    # diagonal-tile causal masks: mask[k, j, q] = 1 if k <= q - 128*j
    kk = np.arange(128)[:, None, None]
    jj = np.arange(c.kpq)[None, :, None]
    qq = np.arange(c.QC)[None, None, :]
    masks_np = (kk <= qq - 128 * jj).astype(NP_F16)  # [128, kpq, QC]
    masks_d = nc.inline_tensor(masks_np, name="masks")

    scale_s = 1.0 / math.sqrt(c.hd)        # main attention scale
    scale_a = 1.0 / math.sqrt(c.n_heads)   # adapter scale (faithful to ref)

    EXP = mybir.ActivationFunctionType.Exp
    ADD = mybir.AluOpType.add
    MUL = mybir.AluOpType.mult

    with tile.TileContext(nc) as tc:
        with (
            tc.tile_pool(name="big", bufs=1) as bigp,
            tc.tile_pool(name="persist", bufs=1) as pp,
            tc.tile_pool(name="work", bufs=3) as wp,
            tc.tile_pool(name="psum", bufs=1, space="PSUM") as psp,
        ):
            if nrep == 0:
                # timing baseline: touch every input (the terminal only ships
                # buffers the NEFF references) but do ~zero device work
                z = pp.tile([1, 128], F16, tag="z")
                for i, t in enumerate((xt_d, wqt_d, wkt_d, wvt_d, wot_d,
                                       apt_d, aptv_d, cos_d, sin_d)):
                    nc.sync.dma_start(z[0:1, 8 * i:8 * i + 8], t[0:1, 0:8])
                zo = pp.tile([1, 64], F16, tag="zo")
                nc.gpsimd.memset(zo[:], 0.0)
                nc.sync.dma_start(out_d[0:1, 0:64], zo[:])
            for _rep in range(nrep):
                # ---------- loads (q/k weights + chunk0 + tables first) ----------
                wqt = pp.tile([128, c.nd, c.dh], F16, tag="wqt")
                wkt = pp.tile([128, c.nd, c.dh], F16, tag="wkt")
                wqt_r = wqt_d.rearrange("(t p) m -> p t m", p=128)
                wkt_r = wkt_d.rearrange("(t p) m -> p t m", p=128)
                xt = bigp.tile([128, c.nd, c.s], F16, tag="big")
                xt_r = xt_d.rearrange("(t p) s -> p t s", p=128)
                # quarter-sliced startup loads: the first proj matmuls only
                # need the first t-slices, so get those on chip ASAP
                nq = min(4, c.nd)
                bounds = [c.nd * i // nq for i in range(nq + 1)]
                cos_t = pp.tile([128, c.s], F16, tag="cos")
                sin_t = pp.tile([128, c.s], F16, tag="sin")
                for i in range(nq):
                    ts = slice(bounds[i], bounds[i + 1])
                    nc.sync.dma_start(wqt[:, ts, :], wqt_r[:, ts, :])
                    nc.sync.dma_start(wkt[:, ts, :], wkt_r[:, ts, :])
                    # chunk-0 x slices ride the otherwise-idle ACT HWDGE
                    # ring: both rings stream the startup-critical 3.5 MB in
                    # parallel (ACT has no compute until ~10 us)
                    nc.scalar.dma_start(xt[:, ts, 0:c.QC], xt_r[:, ts, 0:c.QC])
                nc.sync.dma_start(cos_t[:, 0:c.QC], cos_d[:, 0:c.QC])
                nc.sync.dma_start(sin_t[:, 0:c.QC], sin_d[:, 0:c.QC])
                # dummy exp: pulls the ACT function-table load into the
                # startup DMA window instead of stalling the first real copy
                warm = wp.tile([1, 2], F32, tag="warm", bufs=1)
                nc.vector.memset(warm[:], 0.0)
                nc.scalar.activation(warm[:], warm[:], EXP, scale=1.0)
                masks = pp.tile([128, c.kpq, c.QC], F16, tag="masks")
                nc.sync.dma_start(masks[:], masks_d[:])
                # adapter inputs are tiny (80 KB) but feed adapter_kv early in
                # the PE queue -- they must not trail the 12 MB of bulk loads
                apt = pp.tile([128, c.nd, c.L], F16, tag="apt")
                nc.sync.dma_start(apt[:], apt_d.rearrange("(t p) m -> p t m", p=128))
                aptv = pp.tile([128, c.nd, c.L], F16, tag="aptv")
                nc.sync.dma_start(aptv[:], aptv_d.rearrange("(t p) m -> p t m", p=128))
                wvt = pp.tile([128, c.nd, c.dh], F16, tag="wvt")
                nc.sync.dma_start(wvt[:], wvt_d.rearrange("(t p) m -> p t m", p=128))
                if c.s > c.QC:
                    sl = slice(c.QC, c.s)
                    nc.sync.dma_start(cos_t[:, sl], cos_d[:, sl])
                    nc.sync.dma_start(sin_t[:, sl], sin_d[:, sl])
                for qc in range(1, c.ns):
                    sl = slice(qc * c.QC, (qc + 1) * c.QC)
                    nc.sync.dma_start(xt[:, :, sl], xt_r[:, :, sl])
                wot = pp.tile([128, c.hpc, c.d], F16, tag="wot")
                nc.sync.dma_start(wot[:], wot_d.rearrange("(t p) m -> p t m", p=128))
                # all-ones: column [:, 0:1] is the row-sum lhsT; row slices at
                # 32-aligned base partitions serve the broadcast matmuls
                # (matmul requires lhsT/rhs base partitions equal and 32-aligned)
                ones_t = pp.tile([128, 128], F16, tag="ones_t")
                nc.gpsimd.memset(ones_t[:], 1.0)

                # ---------- persistent intermediates ----------
                qrot = [pp.tile([128, c.s], F16, tag=f"qrot{h}", name=f"qrot{h}")
                        for h in range(c.hpc)]
                krot = [pp.tile([128, c.s], F16, tag=f"krot{h}", name=f"krot{h}")
                        for h in range(c.hpc)]
                v_sb = pp.tile([128, c.nst, c.dh], F16, tag="v")
                akt = pp.tile([128, c.hpc, c.L], F16, tag="akt")
                av_sb = pp.tile([c.L, c.dh], F16, tag="av")

                def rope(dst, src_ps, sl):
                    # dst[0:64]   = src[0:64]*cos[0:64] - src[64:]*sin[0:64]
                    # dst[64:128] = src[64:]*cos[64:]   + src[0:64]*sin[64:]
                    # sin_t is pre-signed on host: rows 0:64 hold -sin.
                    qf = wp.tile([128, c.QC], F16, tag="qf", bufs=4)
                    nc.vector.tensor_copy(qf[:], src_ps[:])
                    # sin_t halves are swapped+signed on host so each TT's
                    # two SBUF inputs share a base partition (HW requirement)
                    t2 = wp.tile([128, c.QC], F16, tag="tmp", bufs=6)
                    nc.vector.tensor_tensor(t2[0:64], qf[64:128],
                                            sin_t[64:128, sl], MUL)
                    nc.vector.tensor_tensor(t2[64:128], qf[0:64],
                                            sin_t[0:64, sl], MUL)
                    t1 = wp.tile([128, c.QC], F16, tag="tmp", bufs=6)
                    nc.vector.tensor_tensor(t1[:], qf[:], cos_t[:, sl], MUL)
                    nc.vector.tensor_tensor(dst, t1[:], t2[:], ADD)

                def proj_chunk(qc):
                    sl = slice(qc * c.QC, (qc + 1) * c.QC)
                    for h in range(c.hpc):
                        hsl = slice(h * 128, (h + 1) * 128)
                        q_ps = psp.tile([128, c.QC], F32, tag="qk", bufs=2)
                        for t in range(c.nd):
                            nc.tensor.matmul(q_ps[:], wqt[:, t, hsl], xt[:, t, sl],
                                             start=(t == 0), stop=(t == c.nd - 1))
                        k_ps = psp.tile([128, c.QC], F32, tag="qk", bufs=2)
                        for t in range(c.nd):
                            nc.tensor.matmul(k_ps[:], wkt[:, t, hsl], xt[:, t, sl],
                                             start=(t == 0), stop=(t == c.nd - 1))
                        rope(qrot[h][:, sl], q_ps, sl)
                        rope(krot[h][:, sl], k_ps, sl)
                    for st in range(c.kpq):
                        gst = qc * c.kpq + st
                        ssl = slice(gst * 128, (gst + 1) * 128)
                        v_ps = psp.tile([128, c.dh], F32, tag="sum", bufs=2)
                        for t in range(c.nd):
                            nc.tensor.matmul(v_ps[:], xt[:, t, ssl], wvt[:, t, :],
                                             start=(t == 0), stop=(t == c.nd - 1))
                        nc.scalar.copy(v_sb[:, gst, :], v_ps[:])

                def adapter_kv():
                    for h in range(c.hpc):
                        hsl = slice(h * 128, (h + 1) * 128)
                        a_ps = psp.tile([128, c.L], F32, tag="sum", bufs=2)
                        for t in range(c.nd):
                            nc.tensor.matmul(a_ps[:], wkt[:, t, hsl], apt[:, t, :],
                                             start=(t == 0), stop=(t == c.nd - 1))
                        nc.scalar.copy(akt[:, h, :], a_ps[:])
                    av_ps = psp.tile([c.L, c.dh], F32, tag="sum", bufs=2)
                    for t in range(c.nd):
                        nc.tensor.matmul(av_ps[:], aptv[:, t, :], wvt[:, t, :],
                                         start=(t == 0), stop=(t == c.nd - 1))
                    nc.scalar.copy(av_sb[:], av_ps[:])

                def attn_scores(qc, state):
                    sl = slice(qc * c.QC, (qc + 1) * c.QC)
                    nkt = qc * c.kpq + c.kpq  # causal: k-tiles 0..nkt-1
                    for h in range(c.hpc):
                        hsl = slice(h * 128, (h + 1) * 128)
                        ctx_ps = psp.tile([128, c.QC], F32, tag="ctx", bufs=2,
                                          name=f"ctx{qc}_{h}")
                        # per-head softmax denominators in one PSUM bank:
                        # main sum at row 0, adapter sum at row 32 (matmul
                        # outputs must start at a 32-aligned partition)
                        sums = psp.tile([33, c.QC], F32, tag="sum", bufs=2,
                                        name=f"sums{qc}_{h}")

                        def st_mm(kt):
                            ksl = slice(kt * 128, (kt + 1) * 128)
                            ps = psp.tile([128, c.QC], F32, tag="st", bufs=2,
                                          name=f"st{qc}_{h}_{kt}")
                            nc.tensor.matmul(ps[:], krot[h][:, ksl],
                                             qrot[h][:, sl], start=True, stop=True)
                            return ps

                        # 2-deep score pre-issue: exp(kt) gets PE-iteration
                        # slack before ctx(kt) needs its result
                        # row sums: accumulate est tiles elementwise on the
                        # (underloaded) DVE at fp16 2x mode, then ONE ones-
                        # matmul partition-reduce -- drops a third of the PE's
                        # per-k-tile streaming cost. acc values stay O(30), so
                        # fp16 accumulation is safe.
                        acc = wp.tile([128, c.QC], F16, tag="acc", bufs=2,
                                      name=f"acc{qc}_{h}")
                        st_q = [st_mm(kt) for kt in range(min(3, nkt))]
                        for i, kt in enumerate(range(nkt)):
                            st_cur = st_q.pop(0)
                            if kt + 3 < nkt:
                                st_q.append(st_mm(kt + 3))
                            est = wp.tile([128, c.QC], F16, tag="est", bufs=10)
                            nc.scalar.activation(est[:], st_cur[:], EXP, scale=scale_s)
                            j = kt - qc * c.kpq
                            if j >= 0:
                                nc.vector.tensor_tensor(est[:], est[:], masks[:, j, :],
                                                        MUL)
                            # ctx accumulation group stays open for the
                            # adapter contribution appended in the combine pass
                            nc.tensor.matmul(ctx_ps[:], v_sb[:, kt, hsl], est[:],
                                             start=(i == 0), stop=False)
                            if i == 0:
                                nc.vector.tensor_copy(acc[:], est[:])
                            else:
                                nc.vector.tensor_tensor(acc[:], acc[:], est[:],
                                                        ADD)
                        nc.tensor.matmul(sums[0:1, :], ones_t[:, 0:1], acc[:],
                                         start=True, stop=True)
                        # adapter attention (no rope on adapter k, scale 1/sqrt(H))
                        ast_ps = psp.tile([c.L, c.QC], F32, tag="st", bufs=2,
                                          name=f"ast{qc}_{h}")
                        nc.tensor.matmul(ast_ps[:], akt[:, h, :], qrot[h][:, sl],
                                         start=True, stop=True)
                        aest = wp.tile([c.L, c.QC], BF16, tag="aest", bufs=2,
                                       name=f"aest{qc}_{h}")
                        nc.scalar.activation(aest[:], ast_ps[:], EXP, scale=scale_a)
                        nc.tensor.matmul(sums[32:33, :], ones_t[0:c.L, 0:1],
                                         aest[:], start=True, stop=True)
                        state[h] = (ctx_ps, sums, aest)

                def attn_combine(qc, state, ct_tiles):
                    # combine: ct = (ctx + actx_g*sum/asum)/sum  (gate is
                    # pre-folded into av via the host-scaled aptv). Emitted
                    # AFTER proj(qc+1) so the DVE reciprocal chain drains
                    # while the PE queue works on projections.
                    for h in range(c.hpc):
                        hsl = slice(h * 128, (h + 1) * 128)
                        ctx_ps, sums, aest = state[h]
                        rall = wp.tile([33, c.QC], F16, tag="rall", bufs=2,
                                       name=f"rall{qc}_{h}")
                        # adapter sums reach ~6e5, so 1/asum is fp16-SUBNORMAL;
                        # that reciprocal must stay fp32 (f2 = sum/asum itself
                        # is fp16-safe). 1/sum is ~1e-4..1 -> fp16 fine.
                        ra32 = wp.tile([1, c.QC], F32, tag="ra32", bufs=2,
                                       name=f"ra32_{qc}_{h}")
                        f2 = wp.tile([1, c.QC], F16, tag="f2", bufs=2,
                                     name=f"f2_{qc}_{h}")
                        with nc.allow_low_precision(reason="1/softmax-sum fp16"):
                            nc.vector.reciprocal(rall[0:1], sums[0:1])
                            nc.vector.reciprocal(ra32[0:1], sums[32:33])
                            # PSUM+SBUF mix is exempt from the equal-base rule
                            nc.vector.tensor_tensor(f2[0:1], sums[0:1],
                                                    ra32[0:1], MUL)
                        f10_ps = psp.tile([c.L, c.QC], F32, tag="st", bufs=2,
                                          name=f"f10_{qc}_{h}")
                        nc.tensor.matmul(f10_ps[:], ones_t[0:1, 0:c.L],
                                         f2[0:1, :], start=True, stop=True)
                        aest2 = wp.tile([c.L, c.QC], F16, tag="aest2", bufs=2,
                                        name=f"aest2_{qc}_{h}")
                        nc.vector.tensor_tensor(aest2[:], aest[:], f10_ps[:], MUL)
                        nc.tensor.matmul(ctx_ps[:], av_sb[:, hsl], aest2[:],
                                         start=False, stop=True)
                        rcb_ps = psp.tile([128, c.QC], F32, tag="st", bufs=2,
                                          name=f"rcb{qc}_{h}")
                        nc.tensor.matmul(rcb_ps[:], ones_t[0:1, :], rall[0:1, :],
                                         start=True, stop=True)
                        rcb = wp.tile([128, c.QC], F16, tag="rcb", bufs=2,
                                      name=f"rcbs{qc}_{h}")
                        nc.scalar.copy(rcb[:], rcb_ps[:])
                        ct = wp.tile([128, c.QC], F16, tag="ct", bufs=6,
                                     name=f"ct{qc}_{h}")
                        nc.vector.tensor_tensor(ct[:], ctx_ps[:], rcb[:], MUL)
                        ct_tiles[h] = ct

                def out_proj(qc, ct_tiles):
                    # out_pT[do, q] += wot[:, h, do].T @ ct[h]  (local heads only;
                    # cross-core reduction happens on the host as the unshard)
                    sl = slice(qc * c.QC, (qc + 1) * c.QC)
                    for dt in range(c.nd):
                        dsl = slice(dt * 128, (dt + 1) * 128)
                        o_ps = psp.tile([128, c.QC], F32, tag="ctx", bufs=2,
                                        name=f"o_ps{qc}_{dt}")
                        for h in range(c.hpc):
                            nc.tensor.matmul(o_ps[:], wot[:, h, dsl], ct_tiles[h][:],
                                             start=(h == 0), stop=(h == c.hpc - 1))
                        o_sb = wp.tile([128, c.QC], F16, tag="osb", bufs=6,
                                       name=f"o_sb{qc}_{dt}")
                        if dt % 2 == 0:
                            nc.scalar.copy(o_sb[:], o_ps[:])
                        else:
                            nc.vector.tensor_copy(o_sb[:], o_ps[:])
                        nc.sync.dma_start(out_d[dsl, sl], o_sb[:])

                # ---------- fused pipeline ----------
                # (emit proj(qc+1) before out_proj(qc): the PE engine queue is
                # FIFO, and out_proj waits on the DVE combine chain -- emitting
                # it first would head-of-line-block the next chunk's matmuls)
                proj_chunk(0)
                adapter_kv()
                for qc in range(c.ns):
                    state, ct_tiles = {}, {}
                    attn_scores(qc, state)
                    if qc + 1 < c.ns:
                        proj_chunk(qc + 1)
                    attn_combine(qc, state, ct_tiles)
                    out_proj(qc, ct_tiles)

    nc.compile()
    return nc


def make_in_maps(cfg, hidden_states, Wq, Wk, Wv, Wo, adaption_prompt,
                 adaption_gate, position_ids):
    """Host-side sharding: slice/transpose/cast per core + RoPE tables."""
    c = cfg
    x = np.asarray(hidden_states, np.float32)[0]          # [s, d]
    xt = np.ascontiguousarray(x.T).astype(NP_F16)         # [d, s]
    ap = np.asarray(adaption_prompt, np.float32)[0]       # [L, d]
    apt = np.ascontiguousarray(ap.T).astype(NP_F16)       # [d, L]
    gate = float(np.asarray(adaption_gate).reshape(-1)[0])
    aptv = np.ascontiguousarray(gate * ap.T).astype(NP_F16)
    # RoPE tables in the [hd, s] transposed layout; sin pre-signed.
    pos = np.asarray(position_ids).reshape(-1).astype(np.float64)  # [s]
    inv = 1.0 / (c.rope_base ** (np.arange(0, c.hd, 2, dtype=np.float64) / c.hd))
    f = inv[:, None] * pos[None, :]                       # [hd/2, s]
    cos_t = np.concatenate([np.cos(f), np.cos(f)], axis=0).astype(NP_F16)
    sv = np.sin(f)
    # halves swapped: rows 0:64 multiply q[0:64] (+sin, lands in dst[64:]),
    # rows 64:128 multiply q[64:128] (-sin, lands in dst[0:64])
    sin_t = np.concatenate([sv, -sv], axis=0).astype(NP_F16)
    in_maps = []
    for i in range(c.n_cores):
        rs = slice(i * c.dh, (i + 1) * c.dh)
        in_maps.append({
            "xt": xt,
            "wqt": np.ascontiguousarray(np.asarray(Wq, np.float32)[rs, :].T).astype(NP_F16),
            "wkt": np.ascontiguousarray(np.asarray(Wk, np.float32)[rs, :].T).astype(NP_F16),
            "wvt": np.ascontiguousarray(np.asarray(Wv, np.float32)[rs, :].T).astype(NP_F16),
            "wot": np.ascontiguousarray(np.asarray(Wo, np.float32)[:, rs].T).astype(NP_F16),
            "apt": apt,
            "aptv": aptv,
            "cost": cos_t,
            "sint": sin_t,
        })
    return in_maps


def assemble_output(cfg, results):
    acc = np.zeros((cfg.d, cfg.s), np.float32)
    for r in results:
        acc += np.asarray(r["out"], np.float32)           # per-core partial [d, s]
    return np.ascontiguousarray(acc.T)[None]              # [1, s, d]


_NC_CACHE = {}


def run(inputs, cfg=None, trace=False):
    cfg = cfg or Cfg()
    key = (cfg.s, cfg.d, cfg.L, cfg.n_cores, cfg.n_heads)
    if key not in _NC_CACHE:
        _NC_CACHE[key] = build(cfg)
    nc = _NC_CACHE[key]
    in_maps = make_in_maps(cfg, **inputs)
    res = run_bass_kernel_spmd(nc, in_maps, core_ids=list(range(cfg.n_cores)),
                               trace=trace)
    out = assemble_output(cfg, res.results)
    return out, res


def kernel(**inputs) -> np.ndarray:
    out, _ = run(inputs)
    return out.astype(np.float32)

